# revision 11
# baseline (speedup 1.0000x reference)
"""Trainium2 Bass kernel for nn_BasicLayer_14705968022124.

Computes, per sample (data-parallel over batch across 8 NeuronCores):
    h  = lrelu(conv3x3(concat(x, skip), W1) + b1)   -> lrelu(conv3x3(h, W2) + b2)
    ds = conv1x1(h, Wds)
    amap = UnoB(tar) * (entropy(softmax(ds)) >= 1e-3)

Conv3x3 is 9 (+9 for the second input-channel block) shifted matmuls on the
tensor engine accumulating in PSUM, with activations laid out [C=128
partitions, H*W free] and zero-padded borders.  The entropy threshold is
equivalent to |ds0-ds1| <= T (T solved from the 2-class entropy equation);
the margin on this problem's data scale is >2.8 in d-space, far beyond any
fp32/f32r rounding, so the compare is exact.

Matmuls run in float32r (TF32-like, ~1.6e-4 rel err, 4x the fp32 rate);
everything else is fp32.
"""
import sys

sys.path.insert(0, "/opt/trn_rl_repo")

import math

import numpy as np

import concourse.bacc as bacc
import concourse.mybir as mybir
import concourse.tile as tile

f32 = mybir.dt.float32
f32r = mybir.dt.float32r
i32 = mybir.dt.int32

N_CORES = 8
B, C, H, W = 8, 128, 128, 128
HP, WP = H + 2, W + 2  # padded
STRIP = 8        # conv1 output rows per input-strip iteration
GROUP = 4        # output rows per PSUM group (4*128 = 512 = max fp32 free)
MM_DT = f32r     # dtype for the big conv matmuls


def _solve_T():
    # solve -(p*log2(p+1e-6) + q*log2(q+1e-6)) = 0.001 with p=sigmoid(-T)
    def ent(d):
        p = 1.0 / (1.0 + math.exp(-d))
        q = 1.0 - p
        return -(p * math.log2(p + 1e-6) + q * math.log2(q + 1e-6))

    lo, hi = 1.0, 50.0
    for _ in range(200):
        mid = 0.5 * (lo + hi)
        if ent(mid) >= 0.001:
            lo = mid
        else:
            hi = mid
    return 0.5 * (lo + hi)


T_THRESH = _solve_T()          # ~9.6355
T2 = float(np.float32(T_THRESH * T_THRESH))


def build_nc(rep=1, mm_dt=MM_DT):
    nc = bacc.Bacc(None, target_bir_lowering=False)

    x_d = nc.dram_tensor("x", [C, H, W], mm_dt, kind="ExternalInput")
    s_d = nc.dram_tensor("skip", [C, H, W], mm_dt, kind="ExternalInput")
    tar_d = nc.dram_tensor("tar", [H, W], i32, kind="ExternalInput")
    wt1_d = nc.dram_tensor("wt1", [128, 2 * 9 * 128], mm_dt, kind="ExternalInput")
    wt2_d = nc.dram_tensor("wt2", [128, 9 * 128], mm_dt, kind="ExternalInput")
    wds_d = nc.dram_tensor("wds", [128, 2], mm_dt, kind="ExternalInput")
    b1_d = nc.dram_tensor("b1", [128, 1], f32, kind="ExternalInput")
    b2_d = nc.dram_tensor("b2", [128, 1], f32, kind="ExternalInput")
    sv_d = nc.dram_tensor("sv", [128, 128], f32, kind="ExternalInput")
    bm_d = nc.dram_tensor("bm", [H, W], f32, kind="ExternalInput")

    h_d = nc.dram_tensor("h", [C, H, W], f32, kind="ExternalOutput")
    ds_d = nc.dram_tensor("ds", [2, H, W], f32, kind="ExternalOutput")
    amap_d = nc.dram_tensor("amap", [H, W], f32, kind="ExternalOutput")

    LR = mybir.ActivationFunctionType.Lrelu
    CP = mybir.ActivationFunctionType.Copy
    OP = mybir.AluOpType

    def r(ap):  # matmul-dtype view
        return ap.bitcast(mm_dt)

    with tile.TileContext(nc) as tc:
        with (
            tc.tile_pool(name="const", bufs=1) as cpool,
            tc.tile_pool(name="io", bufs=3) as iopool,
            tc.tile_pool(name="tail", bufs=1) as tpool,
            tc.tile_pool(name="psum", bufs=6, space="PSUM") as ppool,
            tc.tile_pool(name="psmall", bufs=1, space="PSUM") as pspool,
        ):
            wt1 = cpool.tile([128, 2 * 9 * 128], mm_dt)
            wt2 = cpool.tile([128, 9 * 128], mm_dt)
            wds = cpool.tile([128, 2], mm_dt)
            b1t = cpool.tile([128, 1], f32)
            b2t = cpool.tile([128, 1], f32)
            svt = cpool.tile([128, 128], f32)
            bmt = cpool.tile([H, W], f32)
            tari = cpool.tile([H, W], i32)
            h1 = cpool.tile([128, HP, WP], mm_dt)

            nc.sync.dma_start(wt1[:], wt1_d[:])
            nc.sync.dma_start(wt2[:], wt2_d[:])
            nc.sync.dma_start(wds[:], wds_d[:])
            nc.sync.dma_start(b1t[:], b1_d[:])
            nc.sync.dma_start(b2t[:], b2_d[:])
            nc.sync.dma_start(svt[:], sv_d[:])
            nc.sync.dma_start(bmt[:], bm_d[:])
            nc.sync.dma_start(tari[:], tar_d[:])

            # zero the h1 halo once; interior is fully overwritten each rep
            nc.gpsimd.memset(h1[:, 0:1, :].bitcast(f32), 0.0)
            nc.gpsimd.memset(h1[:, HP - 1 : HP, :].bitcast(f32), 0.0)
            nc.gpsimd.memset(h1[:, :, 0:1].bitcast(f32), 0.0)
            nc.gpsimd.memset(h1[:, :, WP - 1 : WP].bitcast(f32), 0.0)

            for _ in range(rep):
                # ---------------- conv1: (x, skip) -> h1 ----------------
                for s in range(H // STRIP):
                    y0 = s * STRIP
                    xs = iopool.tile([128, STRIP + 2, WP], mm_dt, tag="xs")
                    ss = iopool.tile([128, STRIP + 2, WP], mm_dt, tag="ss")
                    for t in (xs, ss):
                        nc.gpsimd.memset(t[:, :, 0:1].bitcast(f32), 0.0)
                        nc.gpsimd.memset(t[:, :, WP - 1 : WP].bitcast(f32), 0.0)
                    lo = y0 - 1
                    hi = y0 + STRIP + 1
                    clo, chi = max(lo, 0), min(hi, H)
                    r0 = clo - lo
                    if lo < 0:
                        nc.gpsimd.memset(xs[:, 0:1, :].bitcast(f32), 0.0)
                        nc.gpsimd.memset(ss[:, 0:1, :].bitcast(f32), 0.0)
                    if hi > H:
                        nc.gpsimd.memset(xs[:, STRIP + 1 : STRIP + 2, :].bitcast(f32), 0.0)
                        nc.gpsimd.memset(ss[:, STRIP + 1 : STRIP + 2, :].bitcast(f32), 0.0)
                    nc.sync.dma_start(xs[:, r0 : r0 + chi - clo, 1 : W + 1],
                                      x_d[:, clo:chi, :])
                    nc.sync.dma_start(ss[:, r0 : r0 + chi - clo, 1 : W + 1],
                                      s_d[:, clo:chi, :])

                    for g in range(STRIP // GROUP):
                        ps = ppool.tile([128, GROUP * W], f32, tag="ps")
                        k = 0
                        for src in (xs, ss):
                            base = 0 if src is xs else 9 * 128
                            for dy in range(3):
                                for dx in range(3):
                                    tap = dy * 3 + dx
                                    nc.tensor.matmul(
                                        ps[:],
                                        wt1[:, base + tap * 128 : base + (tap + 1) * 128],
                                        src[:, g * GROUP + dy : g * GROUP + dy + GROUP,
                                            dx : dx + W],
                                        start=(k == 0), stop=(k == 17),
                                    )
                                    k += 1
                        nc.scalar.activation(
                            h1[:, 1 + y0 + g * GROUP : 1 + y0 + (g + 1) * GROUP, 1 : W + 1],
                            ps[:], LR, bias=b1t[:], scale=1.0, alpha=0.01)

                # ---------------- conv2 + ds + outputs ----------------
                dsT = tpool.tile([128, 2, W], f32, tag="dsT")
                for g in range(H // GROUP):
                    y0 = g * GROUP
                    ps = ppool.tile([128, GROUP * W], f32, tag="ps")
                    for dy in range(3):
                        for dx in range(3):
                            tap = dy * 3 + dx
                            nc.tensor.matmul(
                                ps[:],
                                wt2[:, tap * 128 : (tap + 1) * 128],
                                h1[:, y0 + dy : y0 + dy + GROUP, dx : dx + W],
                                start=(tap == 0), stop=(tap == 8),
                            )
                    h2s = iopool.tile([128, GROUP, W], mm_dt, tag="h2s")
                    nc.scalar.activation(h2s[:], ps[:], LR,
                                         bias=b2t[:], scale=1.0, alpha=0.01)
                    nc.sync.dma_start(h_d[:, y0 : y0 + GROUP, :], h2s[:].bitcast(f32))

                    psd = pspool.tile([2, GROUP * W], f32, tag="psd")
                    nc.tensor.matmul(psd[:], wds[:], h2s[:],
                                     start=True, stop=True)
                    dss = iopool.tile([2, GROUP, W], f32, tag="dss")
                    nc.scalar.activation(dss[:], psd[:], CP)
                    nc.sync.dma_start(ds_d[:, y0 : y0 + GROUP, :], dss[:])
                    for c in range(2):
                        nc.sync.dma_start(dsT[y0 : y0 + GROUP, c, :],
                                          dss[c : c + 1, :, :])

                # ---------------- amap: threshold + UnoB ----------------
                d2 = tpool.tile([H, W], f32, tag="d2")
                nc.vector.tensor_tensor(d2[:], dsT[:, 0, :], dsT[:, 1, :],
                                        op=OP.subtract)
                sq = tpool.tile([H, W], f32, tag="sq")
                nc.vector.tensor_tensor(sq[:], d2[:], d2[:], op=OP.mult)
                am0 = tpool.tile([H, W], f32, tag="am0")
                nc.vector.tensor_scalar(am0[:], sq[:], T2, None, OP.is_le)

                tf = tpool.tile([H, W], f32, tag="tf")
                nc.vector.tensor_copy(tf[:], tari[:])
                pv = pspool.tile([H, W], f32, tag="pv")
                nc.tensor.matmul(pv[:], svt[:], tf[:], start=True, stop=True)
                hs = tpool.tile([H, W], f32, tag="hs")
                nc.gpsimd.memset(hs[:, 0:1], 0.0)
                nc.gpsimd.memset(hs[:, W - 1 : W], 0.0)
                nc.vector.tensor_tensor(hs[:, 1 : W - 1], tf[:, 0 : W - 2],
                                        tf[:, 2:W], op=OP.add)
                nb = tpool.tile([H, W], f32, tag="nb")
                nc.vector.tensor_tensor(nb[:], pv[:], hs[:], op=OP.add)
                t4 = tpool.tile([H, W], f32, tag="t4")
                nc.vector.tensor_scalar(t4[:], tf[:], 4.0, None, OP.mult)
                eqt = tpool.tile([H, W], f32, tag="eqt")
                nc.vector.tensor_tensor(eqt[:], t4[:], nb[:], op=OP.is_equal)
                cz = tpool.tile([H, W], f32, tag="cz")
                nc.vector.tensor_scalar(cz[:], tf[:], 0.0, None, OP.is_equal)
                pro = tpool.tile([H, W], f32, tag="pro")
                nc.vector.tensor_tensor(pro[:], eqt[:], cz[:], op=OP.max)
                pro2 = tpool.tile([H, W], f32, tag="pro2")
                nc.vector.tensor_tensor(pro2[:], pro[:], bmt[:], op=OP.max)
                amf = tpool.tile([H, W], f32, tag="amf")
                nc.vector.tensor_tensor(amf[:], am0[:], pro2[:], op=OP.mult)
                nc.sync.dma_start(amap_d[:], amf[:])

    nc.finalize()
    return nc


# ---------------------------------------------------------------------------
# persistent-jit SPMD runner (mirrors concourse.bass2jax.run_bass_via_pjrt)
# ---------------------------------------------------------------------------
def make_runner(nc, n_cores):
    import jax
    from jax.experimental.shard_map import shard_map
    from jax.sharding import Mesh, PartitionSpec

    from concourse.bass2jax import (_bass_exec_p, install_neuronx_cc_hook,
                                    partition_id_tensor)

    install_neuronx_cc_hook()
    partition_name = nc.partition_id_tensor.name if nc.partition_id_tensor else None

    in_names, out_names, out_avals, zero_shapes = [], [], [], []
    for alloc in nc.m.functions[0].allocations:
        if not isinstance(alloc, mybir.MemoryLocationSet):
            continue
        name = alloc.memorylocations[0].name
        if alloc.kind == "ExternalInput":
            if name != partition_name:
                in_names.append(name)
        elif alloc.kind == "ExternalOutput":
            shape = tuple(alloc.tensor_shape)
            dtype = mybir.dt.np(alloc.dtype)
            out_names.append(name)
            out_avals.append(jax.core.ShapedArray(shape, dtype))
            zero_shapes.append((shape, dtype))
    n_params = len(in_names)
    n_outs = len(out_avals)
    all_in_names = list(in_names) + list(out_names)
    if partition_name is not None:
        all_in_names.append(partition_name)
    donate = tuple(range(n_params, n_params + n_outs))

    def _body(*args):
        operands = list(args)
        if partition_name is not None:
            operands.append(partition_id_tensor())
        outs = _bass_exec_p.bind(
            *operands,
            out_avals=tuple(out_avals),
            in_names=tuple(all_in_names),
            out_names=tuple(out_names),
            lowering_input_output_aliases=(),
            sim_require_finite=True,
            sim_require_nnan=True,
            nc=nc,
        )
        return tuple(outs)

    devices = jax.devices()[:n_cores]
    assert len(devices) == n_cores, f"need {n_cores} cores, see {len(jax.devices())}"
    mesh = Mesh(np.asarray(devices), ("core",))
    in_specs = (PartitionSpec("core"),) * (n_params + n_outs)
    out_specs = (PartitionSpec("core"),) * n_outs
    sharded = jax.jit(
        shard_map(_body, mesh=mesh, in_specs=in_specs, out_specs=out_specs,
                  check_rep=False),
        donate_argnums=donate,
        keep_unused=True,
    )

    def run(in_maps):
        import jax
        per_core = [[np.asarray(m[name]) for name in in_names] for m in in_maps]
        concat_in = [
            np.concatenate([per_core[c][i] for c in range(n_cores)], axis=0)
            for i in range(n_params)
        ]
        concat_zeros = [np.zeros((n_cores * s[0], *s[1:]), d)
                        for (s, d) in zero_shapes]
        out_arrs = sharded(*concat_in, *concat_zeros)
        jax.block_until_ready(out_arrs)
        return [
            {name: np.asarray(out_arrs[i]).reshape(n_cores, *out_avals[i].shape)[c]
             for i, name in enumerate(out_names)}
            for c in range(n_cores)
        ]

    return run


_CACHE = {}


def _get_runner(rep=1):
    if rep not in _CACHE:
        nc = build_nc(rep=rep)
        _CACHE[rep] = make_runner(nc, N_CORES)
    return _CACHE[rep]


def _prep_maps(x, skip, tar, W1, b1, W2, b2, Wds):
    x = np.ascontiguousarray(np.asarray(x, np.float32))
    skip = np.ascontiguousarray(np.asarray(skip, np.float32))
    tar = np.ascontiguousarray(np.asarray(tar).astype(np.int32))
    W1 = np.asarray(W1, np.float32)
    W2 = np.asarray(W2, np.float32)
    Wds = np.asarray(Wds, np.float32)

    # lhsT layouts: wt[ci, ...tap..., co]
    wt1 = np.ascontiguousarray(
        W1.reshape(128, 2, 128, 9).transpose(2, 1, 3, 0).reshape(128, 2 * 9 * 128))
    wt2 = np.ascontiguousarray(
        W2.reshape(128, 128, 9).transpose(1, 2, 0).reshape(128, 9 * 128))
    wdsT = np.ascontiguousarray(Wds.reshape(2, 128).T)
    b1c = np.ascontiguousarray(np.asarray(b1, np.float32).reshape(128, 1))
    b2c = np.ascontiguousarray(np.asarray(b2, np.float32).reshape(128, 1))
    sv = np.zeros((128, 128), np.float32)
    idx = np.arange(127)
    sv[idx, idx + 1] = 1.0
    sv[idx + 1, idx] = 1.0
    bm = np.zeros((H, W), np.float32)
    bm[0, :] = bm[-1, :] = bm[:, 0] = bm[:, -1] = 1.0

    return [
        {
            "x": x[c], "skip": skip[c], "tar": tar[c, 0],
            "wt1": wt1, "wt2": wt2, "wds": wdsT,
            "b1": b1c, "b2": b2c, "sv": sv, "bm": bm,
        }
        for c in range(N_CORES)
    ]


def kernel(x, skip, tar, W1, b1, W2, b2, Wds):
    in_maps = _prep_maps(x, skip, tar, W1, b1, W2, b2, Wds)
    results = _get_runner(rep=1)(in_maps)
    h = np.stack([results[c]["h"] for c in range(N_CORES)])
    ds = np.stack([results[c]["ds"] for c in range(N_CORES)])
    amap = np.stack([results[c]["amap"] for c in range(N_CORES)])
    return h, ds, amap


# revision 16
# speedup vs baseline: 209.4577x; 209.4577x over previous
"""Trainium2 Bass kernel for nn_BasicLayer_14705968022124.

Computes, per sample (data-parallel over batch across 8 NeuronCores):
    h  = lrelu(conv3x3(concat(x, skip), W1) + b1)   -> lrelu(conv3x3(h, W2) + b2)
    ds = conv1x1(h, Wds)
    amap = UnoB(tar) * (entropy(softmax(ds)) >= 1e-3)

Conv3x3 is 9 (+9 for the second input-channel block) shifted matmuls on the
tensor engine accumulating in PSUM, with activations laid out [C=128
partitions, H*W free] and zero-padded borders.  The entropy threshold is
equivalent to |ds0-ds1| <= T (T solved from the 2-class entropy equation);
the margin on this problem's data scale is >2.8 in d-space, far beyond any
fp32/f32r rounding, so the compare is exact.

Matmuls run in float32r (TF32-like, ~1.6e-4 rel err, 4x the fp32 rate);
everything else is fp32.
"""
import sys

sys.path.insert(0, "/opt/trn_rl_repo")

import math

import numpy as np

import concourse.bacc as bacc
import concourse.mybir as mybir
import concourse.tile as tile

f32 = mybir.dt.float32
f32r = mybir.dt.float32r
i32 = mybir.dt.int32

N_CORES = 8
B, C, H, W = 8, 128, 128, 128
HP, WP = H + 2, W + 2  # padded
STRIP = 8        # conv1 output rows per input-strip iteration
GROUP = 4        # output rows per PSUM group (4*128 = 512 = max fp32 free)
MM_DT = f32r     # dtype for the big conv matmuls


def _solve_T():
    # solve -(p*log2(p+1e-6) + q*log2(q+1e-6)) = 0.001 with p=sigmoid(-T)
    def ent(d):
        p = 1.0 / (1.0 + math.exp(-d))
        q = 1.0 - p
        return -(p * math.log2(p + 1e-6) + q * math.log2(q + 1e-6))

    lo, hi = 1.0, 50.0
    for _ in range(200):
        mid = 0.5 * (lo + hi)
        if ent(mid) >= 0.001:
            lo = mid
        else:
            hi = mid
    return 0.5 * (lo + hi)


T_THRESH = _solve_T()          # ~9.6355
T2 = float(np.float32(T_THRESH * T_THRESH))


def build_nc(rep=1, mm_dt=MM_DT, timing=False):
    """timing=True: outputs land in internal DRAM (same DMA traffic, no
    host transfer) so repeat-delta wall-clock isolates device time."""
    nc = bacc.Bacc(None, target_bir_lowering=False)

    x_d = nc.dram_tensor("x", [C, H, W], mm_dt, kind="ExternalInput")
    s_d = nc.dram_tensor("skip", [C, H, W], mm_dt, kind="ExternalInput")
    tar_d = nc.dram_tensor("tar", [H, W], i32, kind="ExternalInput")
    wt1_d = nc.dram_tensor("wt1", [128, 2 * 9 * 128], mm_dt, kind="ExternalInput")
    wt2_d = nc.dram_tensor("wt2", [128, 9 * 128], mm_dt, kind="ExternalInput")
    wds_d = nc.dram_tensor("wds", [128, 2], mm_dt, kind="ExternalInput")
    b1_d = nc.dram_tensor("b1", [128, 1], f32, kind="ExternalInput")
    b2_d = nc.dram_tensor("b2", [128, 1], f32, kind="ExternalInput")
    sv_d = nc.dram_tensor("sv", [128, 128], f32, kind="ExternalInput")
    bm_d = nc.dram_tensor("bm", [H, W], f32, kind="ExternalInput")

    if timing:
        h_d = nc.dram_tensor("h", [C, H, W], f32)
        ds_d = nc.dram_tensor("ds", [2, H, W], f32)
        amap_d = nc.dram_tensor("amap", [H, W], f32)
        tiny_d = nc.dram_tensor("tiny", [H, W], f32, kind="ExternalOutput")
    else:
        h_d = nc.dram_tensor("h", [C, H, W], f32, kind="ExternalOutput")
        ds_d = nc.dram_tensor("ds", [2, H, W], f32, kind="ExternalOutput")
        amap_d = nc.dram_tensor("amap", [H, W], f32, kind="ExternalOutput")
        tiny_d = None

    LR = mybir.ActivationFunctionType.Lrelu
    CP = mybir.ActivationFunctionType.Copy
    OP = mybir.AluOpType

    def r(ap):  # matmul-dtype view
        return ap.bitcast(mm_dt)

    with tile.TileContext(nc) as tc:
        with (
            tc.tile_pool(name="const", bufs=1) as cpool,
            tc.tile_pool(name="io", bufs=3) as iopool,
            tc.tile_pool(name="tail", bufs=1) as tpool,
            tc.tile_pool(name="psum", bufs=7, space="PSUM") as ppool,
            tc.tile_pool(name="psmall", bufs=1, space="PSUM") as pspool,
        ):
            wt1 = cpool.tile([128, 2 * 9 * 128], mm_dt)
            wt2 = cpool.tile([128, 9 * 128], mm_dt)
            wds = cpool.tile([128, 2], mm_dt)
            b1t = cpool.tile([128, 1], f32)
            b2t = cpool.tile([128, 1], f32)
            svt = cpool.tile([128, 128], f32)
            bmt = cpool.tile([H, W], f32)
            tari = cpool.tile([H, W], i32)
            h1 = cpool.tile([128, HP, WP], mm_dt)

            nc.sync.dma_start(wt1[:], wt1_d[:])
            nc.sync.dma_start(wt2[:], wt2_d[:])
            nc.sync.dma_start(wds[:], wds_d[:])
            nc.sync.dma_start(b1t[:], b1_d[:])
            nc.sync.dma_start(b2t[:], b2_d[:])
            nc.sync.dma_start(svt[:], sv_d[:])
            nc.sync.dma_start(bmt[:], bm_d[:])
            nc.sync.dma_start(tari[:], tar_d[:])

            # zero the h1 halo once; interior is fully overwritten each rep
            nc.gpsimd.memset(h1[:, 0:1, :].bitcast(f32), 0.0)
            nc.gpsimd.memset(h1[:, HP - 1 : HP, :].bitcast(f32), 0.0)
            nc.gpsimd.memset(h1[:, :, 0:1].bitcast(f32), 0.0)
            nc.gpsimd.memset(h1[:, :, WP - 1 : WP].bitcast(f32), 0.0)

            for _ in range(rep):
                # --------- UnoB mask from tar (independent of convs) ---------
                tf = tpool.tile([H, W], f32, tag="tf")
                nc.vector.tensor_copy(tf[:], tari[:])
                pv = pspool.tile([H, W], f32, tag="pk")
                nc.tensor.matmul(pv[:], svt[:], tf[:], start=True, stop=True)
                hs = tpool.tile([H, W], f32, tag="hs")
                nc.gpsimd.memset(hs[:, 0:1], 0.0)
                nc.gpsimd.memset(hs[:, W - 1 : W], 0.0)
                nc.vector.tensor_tensor(hs[:, 1 : W - 1], tf[:, 0 : W - 2],
                                        tf[:, 2:W], op=OP.add)
                nb = tpool.tile([H, W], f32, tag="nb")
                nc.vector.tensor_tensor(nb[:], pv[:], hs[:], op=OP.add)
                t4 = tpool.tile([H, W], f32, tag="t4")
                nc.vector.tensor_scalar(t4[:], tf[:], 4.0, None, OP.mult)
                eqt = tpool.tile([H, W], f32, tag="eqt")
                nc.vector.tensor_tensor(eqt[:], t4[:], nb[:], op=OP.is_equal)
                cz = tpool.tile([H, W], f32, tag="cz")
                nc.vector.tensor_scalar(cz[:], tf[:], 0.0, None, OP.is_equal)
                pro = tpool.tile([H, W], f32, tag="pro")
                nc.vector.tensor_tensor(pro[:], eqt[:], cz[:], op=OP.max)
                pro2 = tpool.tile([H, W], f32, tag="pro2")
                nc.vector.tensor_tensor(pro2[:], pro[:], bmt[:], op=OP.max)

                # ---------------- conv1: (x, skip) -> h1 ----------------
                for s in range(H // STRIP):
                    y0 = s * STRIP
                    xs = iopool.tile([128, STRIP + 2, WP], mm_dt, tag="xs")
                    ss = iopool.tile([128, STRIP + 2, WP], mm_dt, tag="ss")
                    for t in (xs, ss):
                        nc.gpsimd.memset(t[:, :, 0:1].bitcast(f32), 0.0)
                        nc.gpsimd.memset(t[:, :, WP - 1 : WP].bitcast(f32), 0.0)
                    lo = y0 - 1
                    hi = y0 + STRIP + 1
                    clo, chi = max(lo, 0), min(hi, H)
                    r0 = clo - lo
                    if lo < 0:
                        nc.gpsimd.memset(xs[:, 0:1, :].bitcast(f32), 0.0)
                        nc.gpsimd.memset(ss[:, 0:1, :].bitcast(f32), 0.0)
                    if hi > H:
                        nc.gpsimd.memset(xs[:, STRIP + 1 : STRIP + 2, :].bitcast(f32), 0.0)
                        nc.gpsimd.memset(ss[:, STRIP + 1 : STRIP + 2, :].bitcast(f32), 0.0)
                    nc.sync.dma_start(xs[:, r0 : r0 + chi - clo, 1 : W + 1],
                                      x_d[:, clo:chi, :])
                    nc.sync.dma_start(ss[:, r0 : r0 + chi - clo, 1 : W + 1],
                                      s_d[:, clo:chi, :])

                    for g in range(STRIP // GROUP):
                        ps = ppool.tile([128, GROUP * W], f32, tag="ps")
                        k = 0
                        for src in (xs, ss):
                            base = 0 if src is xs else 9 * 128
                            for dy in range(3):
                                for dx in range(3):
                                    tap = dy * 3 + dx
                                    nc.tensor.matmul(
                                        ps[:],
                                        wt1[:, base + tap * 128 : base + (tap + 1) * 128],
                                        src[:, g * GROUP + dy : g * GROUP + dy + GROUP,
                                            dx : dx + W],
                                        start=(k == 0), stop=(k == 17),
                                    )
                                    k += 1
                        nc.scalar.activation(
                            h1[:, 1 + y0 + g * GROUP : 1 + y0 + (g + 1) * GROUP, 1 : W + 1],
                            ps[:], LR, bias=b1t[:], scale=1.0, alpha=0.01)

                # ---------------- conv2 + ds + outputs ----------------
                dsT = tpool.tile([128, 2, W], f32, tag="dsT")
                for g in range(H // GROUP):
                    y0 = g * GROUP
                    ps = ppool.tile([128, GROUP * W], f32, tag="ps")
                    for dy in range(3):
                        for dx in range(3):
                            tap = dy * 3 + dx
                            nc.tensor.matmul(
                                ps[:],
                                wt2[:, tap * 128 : (tap + 1) * 128],
                                h1[:, y0 + dy : y0 + dy + GROUP, dx : dx + W],
                                start=(tap == 0), stop=(tap == 8),
                            )
                    h2s = iopool.tile([128, GROUP, W], mm_dt, tag="h2s")
                    nc.scalar.activation(h2s[:], ps[:], LR,
                                         bias=b2t[:], scale=1.0, alpha=0.01)
                    nc.sync.dma_start(h_d[:, y0 : y0 + GROUP, :], h2s[:].bitcast(f32))

                    psd = pspool.tile([2, GROUP * W], f32, tag="pk")
                    nc.tensor.matmul(psd[:], wds[:], h2s[:],
                                     start=True, stop=True)
                    dss = iopool.tile([2, GROUP, W], f32, tag="dss")
                    nc.scalar.activation(dss[:], psd[:], CP)
                    nc.sync.dma_start(ds_d[:, y0 : y0 + GROUP, :], dss[:])
                    for c in range(2):
                        nc.sync.dma_start(dsT[y0 : y0 + GROUP, c, :],
                                          dss[c : c + 1, :, :])

                # ---------------- amap: threshold * UnoB mask ----------------
                d2 = tpool.tile([H, W], f32, tag="d2")
                nc.vector.tensor_tensor(d2[:], dsT[:, 0, :], dsT[:, 1, :],
                                        op=OP.subtract)
                sq = tpool.tile([H, W], f32, tag="sq")
                nc.vector.tensor_tensor(sq[:], d2[:], d2[:], op=OP.mult)
                am0 = tpool.tile([H, W], f32, tag="am0")
                nc.vector.tensor_scalar(am0[:], sq[:], T2, None, OP.is_le)
                amf = tpool.tile([H, W], f32, tag="amf")
                nc.vector.tensor_tensor(amf[:], am0[:], pro2[:], op=OP.mult)
                nc.sync.dma_start(amap_d[:], amf[:])
            if timing:
                nc.sync.dma_start(tiny_d[:], amf[:])

    nc.finalize()
    return nc


# ---------------------------------------------------------------------------
# persistent-jit SPMD runner (mirrors concourse.bass2jax.run_bass_via_pjrt)
# ---------------------------------------------------------------------------
def make_runner(nc, n_cores):
    import jax
    from jax.experimental.shard_map import shard_map
    from jax.sharding import Mesh, PartitionSpec

    from concourse.bass2jax import (_bass_exec_p, install_neuronx_cc_hook,
                                    partition_id_tensor)

    install_neuronx_cc_hook()
    partition_name = nc.partition_id_tensor.name if nc.partition_id_tensor else None

    in_names, out_names, out_avals, zero_shapes = [], [], [], []
    for alloc in nc.m.functions[0].allocations:
        if not isinstance(alloc, mybir.MemoryLocationSet):
            continue
        name = alloc.memorylocations[0].name
        if alloc.kind == "ExternalInput":
            if name != partition_name:
                in_names.append(name)
        elif alloc.kind == "ExternalOutput":
            shape = tuple(alloc.tensor_shape)
            dtype = mybir.dt.np(alloc.dtype)
            out_names.append(name)
            out_avals.append(jax.core.ShapedArray(shape, dtype))
            zero_shapes.append((shape, dtype))
    n_params = len(in_names)
    n_outs = len(out_avals)
    all_in_names = list(in_names) + list(out_names)
    if partition_name is not None:
        all_in_names.append(partition_name)
    donate = tuple(range(n_params, n_params + n_outs))

    def _body(*args):
        operands = list(args)
        if partition_name is not None:
            operands.append(partition_id_tensor())
        outs = _bass_exec_p.bind(
            *operands,
            out_avals=tuple(out_avals),
            in_names=tuple(all_in_names),
            out_names=tuple(out_names),
            lowering_input_output_aliases=(),
            sim_require_finite=True,
            sim_require_nnan=True,
            nc=nc,
        )
        return tuple(outs)

    devices = jax.devices()[:n_cores]
    assert len(devices) == n_cores, f"need {n_cores} cores, see {len(jax.devices())}"
    mesh = Mesh(np.asarray(devices), ("core",))
    in_specs = (PartitionSpec("core"),) * (n_params + n_outs)
    out_specs = (PartitionSpec("core"),) * n_outs
    sharded = jax.jit(
        shard_map(_body, mesh=mesh, in_specs=in_specs, out_specs=out_specs,
                  check_rep=False),
        donate_argnums=donate,
        keep_unused=True,
    )

    def run(in_maps, device_arrays=None):
        import jax
        if device_arrays is None:
            per_core = [[np.asarray(m[name]) for name in in_names] for m in in_maps]
            concat_in = [
                np.concatenate([per_core[c][i] for c in range(n_cores)], axis=0)
                for i in range(n_params)
            ]
        else:
            concat_in = device_arrays
        concat_zeros = [np.zeros((n_cores * s[0], *s[1:]), d)
                        for (s, d) in zero_shapes]
        out_arrs = sharded(*concat_in, *concat_zeros)
        jax.block_until_ready(out_arrs)
        return [
            {name: np.asarray(out_arrs[i]).reshape(n_cores, *out_avals[i].shape)[c]
             for i, name in enumerate(out_names)}
            for c in range(n_cores)
        ]

    run.in_names = in_names
    return run


_CACHE = {}


def _get_runner(rep=1):
    if rep not in _CACHE:
        nc = build_nc(rep=rep)
        _CACHE[rep] = make_runner(nc, N_CORES)
    return _CACHE[rep]


def _prep_maps(x, skip, tar, W1, b1, W2, b2, Wds):
    x = np.ascontiguousarray(np.asarray(x, np.float32))
    skip = np.ascontiguousarray(np.asarray(skip, np.float32))
    tar = np.ascontiguousarray(np.asarray(tar).astype(np.int32))
    W1 = np.asarray(W1, np.float32)
    W2 = np.asarray(W2, np.float32)
    Wds = np.asarray(Wds, np.float32)

    # lhsT layouts: wt[ci, ...tap..., co]
    wt1 = np.ascontiguousarray(
        W1.reshape(128, 2, 128, 9).transpose(2, 1, 3, 0).reshape(128, 2 * 9 * 128))
    wt2 = np.ascontiguousarray(
        W2.reshape(128, 128, 9).transpose(1, 2, 0).reshape(128, 9 * 128))
    wdsT = np.ascontiguousarray(Wds.reshape(2, 128).T)
    b1c = np.ascontiguousarray(np.asarray(b1, np.float32).reshape(128, 1))
    b2c = np.ascontiguousarray(np.asarray(b2, np.float32).reshape(128, 1))
    sv = np.zeros((128, 128), np.float32)
    idx = np.arange(127)
    sv[idx, idx + 1] = 1.0
    sv[idx + 1, idx] = 1.0
    bm = np.zeros((H, W), np.float32)
    bm[0, :] = bm[-1, :] = bm[:, 0] = bm[:, -1] = 1.0

    return [
        {
            "x": x[c], "skip": skip[c], "tar": tar[c, 0],
            "wt1": wt1, "wt2": wt2, "wds": wdsT,
            "b1": b1c, "b2": b2c, "sv": sv, "bm": bm,
        }
        for c in range(N_CORES)
    ]


def kernel(x, skip, tar, W1, b1, W2, b2, Wds):
    in_maps = _prep_maps(x, skip, tar, W1, b1, W2, b2, Wds)
    results = _get_runner(rep=1)(in_maps)
    h = np.stack([results[c]["h"] for c in range(N_CORES)])
    ds = np.stack([results[c]["ds"] for c in range(N_CORES)])
    amap = np.stack([results[c]["amap"] for c in range(N_CORES)])
    return h, ds, amap
